# revision 9
# baseline (speedup 1.0000x reference)
"""CompressedSparseAttention Trainium2 kernel v2 (8 NeuronCores).

Sharding: data-parallel over batch (2) x tensor-parallel over head-pairs (4).
Core c handles batch b = c//4 and heads (2g, 2g+1) with g = c%4.
Each core computes its partial output attn_out[:, hslice] @ wo[:, hslice].T
([2048, 512] bf16); the host sums the 4 partials per batch in f32.

v2 design notes (vs baseline):
  - all matmul operands bf16 (1 cyc/col on PE)
  - rope swap via host-permuted extra weight streams (wqrot/wkrot), rope
    combine on DVE (no gpsimd partition copies)
  - compressed x_c pooled on host (tiny linear prep, like the x transpose)
  - causal masks via precomputed 0/1 bf16 mask tiles * DVE multiply
    (replaces gpsimd affine_select)
  - AV in [q, dims] layout: av[q, 0:65|65:130] accumulates
    exp(scores)^T @ [v | ones]; col 64/129 = softmax denominators;
    sink contribution via a [1-contraction] outer-product matmul
  - normalization via DVE reciprocal + per-partition scale, then PE
    transpose -> single 128-contraction wo matmul per q-chunk
  - software-pipelined emission: scores/exp of block qb+1 are emitted
    before the AV of block qb (engine streams are in-order)
  - Act engine runs exp ONLY (one activation table, no reloads)
"""

import math

import numpy as np

import concourse.bass as bass
import concourse.mybir as mybir
import concourse.tile as tile
from concourse import bacc
from concourse.bass import ds
from concourse.masks import make_identity

B = 2
L = 2048
D = 512
H = 8
HD = 64
RATIO = 8
STRIDE = 4
WINDOW = 128
THETA = 10000.0
LC = (L - RATIO) // STRIDE + 1  # 511
NCORES = 8
NB = L // 512  # 4 q-blocks of 512
NCH = L // 128  # 16 q-chunks of 128
KD = D // 128  # 4 contraction chunks

F32 = mybir.dt.float32
F32R = mybir.dt.float32r
BF16 = mybir.dt.bfloat16
AF = mybir.ActivationFunctionType
ALU = mybir.AluOpType

_CACHE = {}

# weight column offsets inside wst tiles [128, 896]
WQ, WQR, WK, WKR, WV, WKC, WVC = 0, 128, 256, 384, 512, 640, 768
# const column offsets inside mega tile [128, 6144]
COS0, SIN0, MWIN, MD0, MD1, WOT = 0, 2048, 4096, 4608, 5120, 5632


def _build_nc():
    nc = bacc.Bacc(
        "TRN2",
        target_bir_lowering=False,
        debug=False,
        num_devices=NCORES,
        name="csa2",
    )

    xT_d = nc.dram_tensor("xT", [D, L], BF16, kind="ExternalInput")
    xcT_d = nc.dram_tensor("xcT", [D, LC], BF16, kind="ExternalInput")
    wst_d = nc.dram_tensor("wst", [D, 896], BF16, kind="ExternalInput")
    mega_d = nc.dram_tensor("mega", [128, 6144], BF16, kind="ExternalInput")
    sinkrow_d = nc.dram_tensor("sinkrow", [1, 130], BF16, kind="ExternalInput")
    outp_d = nc.dram_tensor("outp", [L, D], BF16, kind="ExternalOutput")

    with tile.TileContext(nc) as tc:
        with tc.tile_pool(name="consts", bufs=1) as cp, \
             tc.tile_pool(name="work", bufs=1) as wp, \
             tc.tile_pool(name="ps", bufs=4, space="PSUM") as pp, \
             tc.tile_pool(name="pss", bufs=4, space="PSUM") as pps:

            # ---------------- DMAs (in arrival-priority order) ----------
            # one strided DMA per logical tensor: dest packs the 4 dmodel
            # chunks side-by-side along the free dim
            wstt = cp.tile([128, 3584], BF16, tag="wstt")
            wrr = wst_d.rearrange("(c p) f -> p c f", c=KD)
            wtr = wstt.rearrange("p (c f) -> p c f", c=KD)
            nc.sync.dma_start(out=wtr[:, :, 640:896], in_=wrr[:, :, 640:896])
            wst = [wstt[:, ds(896 * c, 896)] for c in range(KD)]

            mega = cp.tile([128, 6144], BF16, tag="mega")
            sinkrow = cp.tile([1, 130], BF16, tag="sinkrow")
            xq = []
            xct = cp.tile([128, 4 * LC], BF16, tag="xct")
            nc.sync.dma_start(
                out=xct,
                in_=xcT_d.rearrange("(c p) f -> p c f", c=KD),
            )
            nc.sync.dma_start(out=wtr[:, :, 0:640], in_=wrr[:, :, 0:640])
            xrr = xT_d.rearrange("(c p) (b f) -> p b c f", c=KD, b=NB)
            for qb in range(NB):
                xt = cp.tile([128, 2048], BF16, tag=f"xq{qb}", name=f"xq{qb}")
                nc.sync.dma_start(out=xt, in_=xrr[:, qb])
                xq.append(xt)
                if qb == 0:
                    nc.sync.dma_start(out=mega[:, 0:4096],
                                      in_=mega_d[:, 0:4096])
                elif qb == 1:
                    nc.sync.dma_start(out=mega[:, 4096:6144],
                                      in_=mega_d[:, 4096:6144])
                    nc.sync.dma_start(out=sinkrow, in_=sinkrow_d[:, :])
            xB = [[xq[qb][:, ds(512 * c, 512)] for qb in range(NB)]
                  for c in range(KD)]
            x_cT = [xct[:, ds(LC * c, LC)] for c in range(KD)]

            onesrow = cp.tile([1, 128], BF16, tag="onesrow")
            nc.gpsimd.memset(onesrow, 1.0)
            ident_bf = cp.tile([128, 128], BF16, tag="ident_bf")
            make_identity(nc, ident_bf)

            cosT = mega[:, COS0:COS0 + L]
            sinST = mega[:, SIN0:SIN0 + L]
            mwin = mega[:, MWIN:MWIN + 512]
            md0 = mega[:, MD0:MD0 + 512]
            md1 = mega[:, MD1:MD1 + 512]
            woT = mega[:, WOT:WOT + 512]

            # ---------------- emission helpers ----------------
            qT = cp.tile([128, L], BF16, tag="qT")
            kT = cp.tile([128, L], BF16, tag="kT")

            def emit_proj_qk(qb):
                for woff, wroff, outT in ((WQ, WQR, qT), (WK, WKR, kT)):
                    ps = pp.tile([128, 512], F32, tag="bank", name="proj_ps")
                    for c in range(KD):
                        nc.tensor.matmul(
                            ps,
                            wst[c][:, ds(woff, 128)],
                            xB[c][qb],
                            start=(c == 0),
                            stop=(c == KD - 1),
                        )
                    psr = pp.tile([128, 512], F32, tag="bank", name="projr_ps")
                    for c in range(KD):
                        nc.tensor.matmul(
                            psr,
                            wst[c][:, ds(wroff, 128)],
                            xB[c][qb],
                            start=(c == 0),
                            stop=(c == KD - 1),
                        )
                    m1 = wp.tile([128, 512], BF16, tag="m1", bufs=2, name="m1")
                    nc.vector.tensor_mul(m1, ps, cosT[:, ds(512 * qb, 512)])
                    m2 = wp.tile([128, 512], BF16, tag="m2", bufs=2, name="m2")
                    nc.vector.tensor_mul(m2, psr, sinST[:, ds(512 * qb, 512)])
                    nc.vector.tensor_add(outT[:, ds(512 * qb, 512)], m1, m2)

            v_aug = [None] * NCH

            def emit_v(qb):
                # vT [vdim, pos] for this 512-block, then 4 transposes
                vt_ps = pp.tile([128, 512], F32, tag="bank", name="vt_ps")
                for c in range(KD):
                    nc.tensor.matmul(
                        vt_ps,
                        wst[c][:, ds(WV, 128)],
                        xB[c][qb],
                        start=(c == 0),
                        stop=(c == KD - 1),
                    )
                vt_sb = wp.tile([128, 512], BF16, tag="vt_sb", bufs=2,
                                name="vt_sb")
                nc.scalar.copy(out=vt_sb, in_=vt_ps)
                for sub in range(4):
                    ch = 4 * qb + sub
                    va = cp.tile([128, 130], BF16, tag=f"v_aug{ch}",
                                 name=f"v_aug{ch}")
                    vag = va.rearrange("p (g c) -> p g c", c=65)
                    nc.vector.memset(vag[:, :, 64], 1.0)
                    tp = pps.tile([128, 128], BF16, tag="small", name="v_tp")
                    nc.tensor.transpose(tp, vt_sb[:, ds(128 * sub, 128)],
                                        ident_bf)
                    if sub % 2 == 0:
                        nc.vector.tensor_copy(out=va[:, 0:64], in_=tp[:, 0:64])
                        nc.vector.tensor_copy(out=va[:, 65:129],
                                              in_=tp[:, 64:128])
                    else:
                        nc.scalar.copy(out=va[:, 0:64], in_=tp[:, 0:64])
                        nc.scalar.copy(out=va[:, 65:129], in_=tp[:, 64:128])
                    v_aug[ch] = va

            k_cT = cp.tile([128, LC], BF16, tag="k_cT")
            vc_aug = [None] * 4

            def emit_kcvc():
                kc_ps = pp.tile([128, LC], F32, tag="bank", name="kc_ps")
                for d in range(KD):
                    nc.tensor.matmul(
                        kc_ps,
                        wst[d][:, ds(WKC, 128)],
                        x_cT[d],
                        start=(d == 0),
                        stop=(d == KD - 1),
                    )
                nc.vector.tensor_copy(out=k_cT, in_=kc_ps)
                vc_ps = pp.tile([128, LC], F32, tag="bank", name="vc_ps")
                for d in range(KD):
                    nc.tensor.matmul(
                        vc_ps,
                        wst[d][:, ds(WVC, 128)],
                        x_cT[d],
                        start=(d == 0),
                        stop=(d == KD - 1),
                    )
                v_cT = wp.tile([128, LC], BF16, tag="v_cT", bufs=1,
                               name="v_cT")
                nc.vector.tensor_copy(out=v_cT, in_=vc_ps)
                for ch in range(4):
                    wlen = min(128, LC - 128 * ch)  # 128,128,128,127
                    va = cp.tile([128, 130], BF16, tag=f"vc_aug{ch}",
                                 name=f"vc_aug{ch}")
                    vag = va.rearrange("p (g c) -> p g c", c=65)
                    nc.vector.memset(vag[:, :, 64], 1.0)
                    tp = pps.tile([128, 128], BF16, tag="small", name="vc_tp")
                    nc.tensor.transpose(tp[0:wlen, :],
                                        v_cT[:, ds(128 * ch, wlen)], ident_bf)
                    nc.scalar.copy(out=va[0:wlen, 0:64], in_=tp[0:wlen, 0:64])
                    nc.scalar.copy(out=va[0:wlen, 65:129],
                                   in_=tp[0:wlen, 64:128])
                    vc_aug[ch] = va

            def emit_scores(qb):
                exc = [[None] * (qb + 1) for _ in range(2)]
                exw = [[None, None] for _ in range(2)]
                brs = [None, None]
                for h in range(2):
                    hs = 64 * h
                    qs = qT[ds(hs, 64), ds(512 * qb, 512)]
                    for wc in range(qb + 1):
                        wlen = min(128, LC - 128 * wc)
                        sc = pp.tile([128, 512], F32, tag="bank", name="sc_ps")
                        nc.tensor.matmul(
                            sc[0:wlen, :],
                            k_cT[ds(hs, 64), ds(128 * wc, wlen)],
                            qs,
                            start=True,
                            stop=True,
                        )
                        ex = wp.tile([128, 512], BF16, tag="exc", bufs=20,
                                     name="exc")
                        if wc >= qb - 1:
                            exr = wp.tile([128, 512], BF16, tag="exr", bufs=3,
                                          name="exr")
                            nc.scalar.activation(
                                out=exr[0:wlen, :], in_=sc[0:wlen, :],
                                func=AF.Exp, scale=0.125,
                            )
                            msk = md0 if wc == qb else md1
                            nc.vector.tensor_mul(
                                ex[0:wlen, :], exr[0:wlen, :], msk[0:wlen, :]
                            )
                        else:
                            nc.scalar.activation(
                                out=ex[0:wlen, :], in_=sc[0:wlen, :],
                                func=AF.Exp, scale=0.125,
                            )
                        exc[h][wc] = ex

                    for p in range(2):
                        kc0 = 4 * qb + 2 * p
                        ncols = 512 if kc0 < 14 else 384
                        wps = pp.tile([128, 512], F32, tag="bank", name="win_ps")
                        for j in range(2):
                            kc = kc0 + j
                            qcols = min(256, L - 128 * kc)
                            nc.tensor.matmul(
                                wps[:, ds(256 * j, qcols)],
                                kT[ds(hs, 64), ds(128 * kc, 128)],
                                qT[ds(hs, 64), ds(128 * kc, qcols)],
                                start=True,
                                stop=True,
                                skip_group_check=True,
                            )
                        ewr = wp.tile([128, 512], BF16, tag="ewr", bufs=3,
                                      name="ewr")
                        nc.scalar.activation(
                            out=ewr[:, 0:ncols], in_=wps[:, 0:ncols],
                            func=AF.Exp, scale=0.125,
                        )
                        ew = wp.tile([128, 512], BF16, tag="exw", bufs=16,
                                     name="exw")
                        nc.vector.tensor_mul(
                            ew[:, 0:ncols], ewr[:, 0:ncols], mwin[:, 0:ncols]
                        )
                        exw[h][p] = ew

                    if qb >= 1:
                        # bridge: keys chunk 4qb-1 vs q-chunk 4qb (replaces
                        # the cross-qb prev_exw handoff)
                        kc = 4 * qb - 1
                        bps = pps.tile([128, 128], F32, tag="small", name="bps")
                        nc.tensor.matmul(
                            bps,
                            kT[ds(hs, 64), ds(128 * kc, 128)],
                            qT[ds(hs, 64), ds(128 * (kc + 1), 128)],
                            start=True,
                            stop=True,
                        )
                        bre = wp.tile([128, 128], BF16, tag="bre", bufs=2,
                                      name="bre")
                        nc.scalar.activation(
                            out=bre, in_=bps, func=AF.Exp, scale=0.125,
                        )
                        br = wp.tile([128, 128], BF16, tag="br", bufs=4,
                                     name="br")
                        nc.vector.tensor_mul(br, bre, mwin[:, 128:256])
                        brs[h] = br
                return exc, exw, brs

            def emit_av_mms(qb, sub, exc, exw, brs):
                c = 4 * qb + sub
                av = pps.tile([128, 130], F32, tag="small", name="av")
                nc.tensor.matmul(
                    av, onesrow, sinkrow, start=True, stop=False,
                    skip_group_check=True,
                )
                for h in range(2):
                    last_h = (h == 1)
                    cols = ds(65 * h, 65)
                    pcur, jcur = sub // 2, sub % 2
                    nc.tensor.matmul(
                        av[:, cols],
                        exw[h][pcur][:, ds(256 * jcur, 128)],
                        v_aug[c][:, cols],
                        start=False, stop=False,
                        skip_group_check=True,
                    )
                    if c > 0:
                        if sub == 0:
                            epv = brs[h][:, 0:128]
                        else:
                            ew_, jpv = exw[h][(sub - 1) // 2], (sub - 1) % 2
                            epv = ew_[:, ds(256 * jpv + 128, 128)]
                        nc.tensor.matmul(
                            av[:, cols],
                            epv,
                            v_aug[c - 1][:, cols],
                            start=False, stop=False,
                            skip_group_check=True,
                        )
                    for wc in range(qb + 1):
                        wlen = min(128, LC - 128 * wc)
                        nc.tensor.matmul(
                            av[:, cols],
                            exc[h][wc][0:wlen, ds(128 * sub, 128)],
                            vc_aug[wc][0:wlen, cols],
                            start=False,
                            stop=(last_h and wc == qb),
                            skip_group_check=True,
                        )
                return av

            outsb = [None] * NB

            def emit_av_chain(av, c):
                avg = av.rearrange("p (g c) -> p g c", c=65)
                rec2 = wp.tile([128, 2], F32, tag="rec2", bufs=3, name="rec2")
                nc.vector.reciprocal(out=rec2, in_=avg[:, :, 64])
                avn = wp.tile([128, 128], BF16, tag="avn", bufs=3, name="avn")
                for h in range(2):
                    nc.vector.tensor_scalar(
                        out=avn[:, ds(64 * h, 64)],
                        in0=av[:, ds(65 * h, 64)],
                        scalar1=rec2[:, ds(h, 1)],
                        scalar2=None,
                        op0=ALU.mult,
                    )
                trp = pps.tile([128, 128], BF16, tag="small", name="trp")
                nc.tensor.transpose(trp, avn, ident_bf)
                avT = wp.tile([128, 128], BF16, tag="avT", bufs=2, name="avT")
                if c % 2 == 0:
                    nc.vector.tensor_copy(out=avT, in_=trp)
                else:
                    nc.scalar.copy(out=avT, in_=trp)
                po = pp.tile([128, 512], F32, tag="bank", name="po")
                nc.tensor.matmul(po, avT, woT, start=True, stop=True)
                qb, sub = c // 4, c % 4
                if sub == 0:
                    outsb[qb] = wp.tile([128, 2048], BF16, tag="osb", bufs=2,
                                        name="osb")
                dst = outsb[qb][:, ds(512 * sub, 512)]
                if sub % 2 == 0:
                    nc.scalar.copy(out=dst, in_=po)
                else:
                    nc.vector.tensor_copy(out=dst, in_=po)
                if sub == 3:
                    nc.sync.dma_start(
                        out=outp_d.rearrange("(b s p) f -> p b s f",
                                             b=NB, s=4)[:, qb],
                        in_=outsb[qb],
                    )

            def emit_av_block(qb, exc, exw, brs):
                avs = [None] * 4
                for sub in range(4):
                    avs[sub] = emit_av_mms(qb, sub, exc, exw, brs)
                    if sub >= 2:
                        emit_av_chain(avs[sub - 2], 4 * qb + sub - 2)
                emit_av_chain(avs[2], 4 * qb + 2)
                emit_av_chain(avs[3], 4 * qb + 3)

            # ---------------- main emission (qb-major) -------------------
            emit_kcvc()
            emit_proj_qk(0)
            emit_v(0)
            tiles = {}
            for qb in range(NB):
                tiles[qb] = emit_scores(qb)
                if qb >= 1:
                    exc, exw, brs = tiles.pop(qb - 1)
                    emit_av_block(qb - 1, exc, exw, brs)
                if qb + 1 < NB:
                    emit_proj_qk(qb + 1)
                    emit_v(qb + 1)
            exc, exw, brs = tiles.pop(NB - 1)
            emit_av_block(NB - 1, exc, exw, brs)

    nc.compile()
    return nc


def _host_prep(inputs):
    """Build the 8 per-core input maps from full inputs."""
    import ml_dtypes

    bf = ml_dtypes.bfloat16
    x = np.asarray(inputs["x"], dtype=np.float32)
    wq = np.asarray(inputs["wq"], dtype=np.float32)
    wk = np.asarray(inputs["wk"], dtype=np.float32)
    wv = np.asarray(inputs["wv"], dtype=np.float32)
    wo = np.asarray(inputs["wo"], dtype=np.float32)
    wk_c = np.asarray(inputs["wk_c"], dtype=np.float32)
    wv_c = np.asarray(inputs["wv_c"], dtype=np.float32)
    gate_logits = np.asarray(inputs["gate_logits"], dtype=np.float32)
    sink_logit = np.asarray(inputs["sink_logit"], dtype=np.float32)

    # rope tables
    half = HD // 2
    inv_freq = 1.0 / (THETA ** (np.arange(half, dtype=np.float32) / half))
    t = np.arange(L, dtype=np.float32)
    f = t[:, None] * inv_freq[None, :]  # [L, 32]
    cos32 = np.cos(f).T.astype(np.float32)  # [32, L]
    sin32 = np.sin(f).T.astype(np.float32)
    cosT = np.tile(cos32, (4, 1))  # rows: i%32
    sinST = np.concatenate([-sin32, sin32, -sin32, sin32], axis=0)

    # masks
    qr = np.arange(512)
    kr = np.arange(128)
    low = (qr[None, 0:128] >= kr[:, None]).astype(np.float32)   # cur chunk
    upp = (kr[:, None] > qr[None, 0:128]).astype(np.float32)    # prev chunk
    mwin = np.concatenate([low, upp, low, upp], axis=1)  # [128, 512]
    md0 = (qr[None, :] >= 4 * kr[:, None] + 7).astype(np.float32)
    md1 = (qr[None, :] >= 4 * kr[:, None] - 505).astype(np.float32)

    mega = np.concatenate(
        [cosT, sinST, mwin, md0, md1, np.zeros((128, 512), np.float32)],
        axis=1,
    )  # [128, 6144]; wo slice filled per core

    # compressed pooling on host: x_c[b, w] = sum_r g_r x[b, 4w+r]
    g = np.exp(gate_logits - gate_logits.max())
    g = (g / g.sum()).astype(np.float32)
    idx = np.arange(RATIO)[None, :] + np.arange(LC)[:, None] * STRIDE  # [LC, 8]
    x_c = np.einsum("bwrd,r->bwd", x[:, idx, :], g)  # [B, LC, D]

    # rope permutation of projection output dims (within the 128-dim slice):
    # row j of the rotated weights = row swap(j): 0..31 <-> 32..63 per head
    perm = np.arange(128).reshape(2, 2, 32)[:, ::-1, :].reshape(128)

    in_maps = []
    for core in range(NCORES):
        b, grp = divmod(core, 4)
        sl = slice(128 * grp, 128 * (grp + 1))
        m = mega.copy()
        m[:, WOT:WOT + 512] = wo[:, sl].T
        wq_s = wq[sl]
        wk_s = wk[sl]
        wst = np.concatenate(
            [wq_s.T, wq_s[perm].T, wk_s.T, wk_s[perm].T,
             wv[sl].T, wk_c[sl].T, wv_c[sl].T], axis=1
        )  # [512, 896]
        sinkrow = np.zeros((1, 130), np.float32)
        sinkrow[0, 64] = np.exp(sink_logit[2 * grp, 0])
        sinkrow[0, 129] = np.exp(sink_logit[2 * grp + 1, 0])
        in_maps.append(
            {
                "xT": np.ascontiguousarray(x[b].T).astype(bf),
                "xcT": np.ascontiguousarray(x_c[b].T).astype(bf),
                "wst": np.ascontiguousarray(wst).astype(bf),
                "mega": m.astype(bf),
                "sinkrow": sinkrow.astype(bf),
            }
        )
    return in_maps


def kernel(**inputs) -> np.ndarray:
    from concourse.bass_utils import run_bass_kernel_spmd

    if "nc" not in _CACHE:
        _CACHE["nc"] = _build_nc()
    nc = _CACHE["nc"]

    in_maps = _host_prep(inputs)
    res = run_bass_kernel_spmd(nc, in_maps, core_ids=list(range(NCORES)))
    out = np.zeros((B, L, D), dtype=np.float32)
    for core in range(NCORES):
        b = core // 4
        out[b] += np.asarray(res.results[core]["outp"], dtype=np.float32)
    return out


# revision 10
# speedup vs baseline: 1.0313x; 1.0313x over previous
"""CompressedSparseAttention Trainium2 kernel v2 (8 NeuronCores).

Sharding: data-parallel over batch (2) x tensor-parallel over head-pairs (4).
Core c handles batch b = c//4 and heads (2g, 2g+1) with g = c%4.
Each core computes its partial output attn_out[:, hslice] @ wo[:, hslice].T
([2048, 512] bf16); the host sums the 4 partials per batch in f32.

v2 design notes (vs baseline):
  - all matmul operands bf16 (1 cyc/col on PE)
  - rope swap via host-permuted extra weight streams (wqrot/wkrot), rope
    combine on DVE (no gpsimd partition copies)
  - compressed x_c pooled on host (tiny linear prep, like the x transpose)
  - causal masks via precomputed 0/1 bf16 mask tiles * DVE multiply
    (replaces gpsimd affine_select)
  - AV in [q, dims] layout: av[q, 0:65|65:130] accumulates
    exp(scores)^T @ [v | ones]; col 64/129 = softmax denominators;
    sink contribution via a [1-contraction] outer-product matmul
  - normalization via DVE reciprocal + per-partition scale, then PE
    transpose -> single 128-contraction wo matmul per q-chunk
  - software-pipelined emission: scores/exp of block qb+1 are emitted
    before the AV of block qb (engine streams are in-order)
  - Act engine runs exp ONLY (one activation table, no reloads)
"""

import math

import numpy as np

import concourse.bass as bass
import concourse.mybir as mybir
import concourse.tile as tile
from concourse import bacc
from concourse.bass import ds
from concourse.masks import make_identity

B = 2
L = 2048
D = 512
H = 8
HD = 64
RATIO = 8
STRIDE = 4
WINDOW = 128
THETA = 10000.0
LC = (L - RATIO) // STRIDE + 1  # 511
NCORES = 8
NB = L // 512  # 4 q-blocks of 512
NCH = L // 128  # 16 q-chunks of 128
KD = D // 128  # 4 contraction chunks

F32 = mybir.dt.float32
F32R = mybir.dt.float32r
BF16 = mybir.dt.bfloat16
AF = mybir.ActivationFunctionType
ALU = mybir.AluOpType

_CACHE = {}

# weight column offsets inside wst tiles [128, 896]
WQ, WQR, WK, WKR, WV, WKC, WVC = 0, 128, 256, 384, 512, 640, 768
# const column offsets inside mega tile [128, 6144]
COS0, SIN0, MWIN, MD0, MD1, WOT = 0, 2048, 4096, 4608, 5120, 5632


def _build_nc():
    nc = bacc.Bacc(
        "TRN2",
        target_bir_lowering=False,
        debug=False,
        num_devices=NCORES,
        name="csa2",
    )

    xT_d = nc.dram_tensor("xT", [D, L], BF16, kind="ExternalInput")
    xcT_d = nc.dram_tensor("xcT", [D, LC], BF16, kind="ExternalInput")
    wst_d = nc.dram_tensor("wst", [D, 896], BF16, kind="ExternalInput")
    mega_d = nc.dram_tensor("mega", [128, 6144], BF16, kind="ExternalInput")
    sinkrow_d = nc.dram_tensor("sinkrow", [1, 130], BF16, kind="ExternalInput")
    outp_d = nc.dram_tensor("outp", [L, D], BF16, kind="ExternalOutput")

    with tile.TileContext(nc) as tc:
        with tc.tile_pool(name="consts", bufs=1) as cp, \
             tc.tile_pool(name="work", bufs=1) as wp, \
             tc.tile_pool(name="ps", bufs=4, space="PSUM") as pp, \
             tc.tile_pool(name="pss", bufs=4, space="PSUM") as pps:

            # ---------------- DMAs (in arrival-priority order) ----------
            # one strided DMA per logical tensor: dest packs the 4 dmodel
            # chunks side-by-side along the free dim
            wstt = cp.tile([128, 3584], BF16, tag="wstt")
            wrr = wst_d.rearrange("(c p) f -> p c f", c=KD)
            wtr = wstt.rearrange("p (c f) -> p c f", c=KD)
            nc.sync.dma_start(out=wtr[:, :, 640:896], in_=wrr[:, :, 640:896])
            wst = [wstt[:, ds(896 * c, 896)] for c in range(KD)]

            mega = cp.tile([128, 6144], BF16, tag="mega")
            sinkrow = cp.tile([1, 130], BF16, tag="sinkrow")
            xq = []
            xct = cp.tile([128, 4 * LC], BF16, tag="xct")
            nc.sync.dma_start(
                out=xct,
                in_=xcT_d.rearrange("(c p) f -> p c f", c=KD),
            )
            nc.sync.dma_start(out=wtr[:, :, 0:640], in_=wrr[:, :, 0:640])
            xrr = xT_d.rearrange("(c p) (b f) -> p b c f", c=KD, b=NB)
            for qb in range(NB):
                xt = cp.tile([128, 2048], BF16, tag=f"xq{qb}", name=f"xq{qb}")
                nc.sync.dma_start(out=xt, in_=xrr[:, qb])
                xq.append(xt)
                if qb == 0:
                    nc.sync.dma_start(out=mega[:, 0:4096],
                                      in_=mega_d[:, 0:4096])
                elif qb == 1:
                    nc.sync.dma_start(out=mega[:, 4096:6144],
                                      in_=mega_d[:, 4096:6144])
                    nc.sync.dma_start(out=sinkrow, in_=sinkrow_d[:, :])
            xB = [[xq[qb][:, ds(512 * c, 512)] for qb in range(NB)]
                  for c in range(KD)]
            x_cT = [xct[:, ds(LC * c, LC)] for c in range(KD)]

            onesrow = cp.tile([1, 128], BF16, tag="onesrow")
            nc.gpsimd.memset(onesrow, 1.0)
            ident_bf = cp.tile([128, 128], BF16, tag="ident_bf")
            make_identity(nc, ident_bf)

            cosT = mega[:, COS0:COS0 + L]
            sinST = mega[:, SIN0:SIN0 + L]
            mwin = mega[:, MWIN:MWIN + 512]
            md0 = mega[:, MD0:MD0 + 512]
            md1 = mega[:, MD1:MD1 + 512]
            woT = mega[:, WOT:WOT + 512]

            # ---------------- emission helpers ----------------
            qT = cp.tile([128, L], BF16, tag="qT")
            kT = cp.tile([128, L], BF16, tag="kT")

            def emit_proj_qk(qb):
                for woff, wroff, outT in ((WQ, WQR, qT), (WK, WKR, kT)):
                    ps = pp.tile([128, 512], F32, tag="bank", name="proj_ps")
                    for c in range(KD):
                        nc.tensor.matmul(
                            ps,
                            wst[c][:, ds(woff, 128)],
                            xB[c][qb],
                            start=(c == 0),
                            stop=(c == KD - 1),
                        )
                    psr = pp.tile([128, 512], F32, tag="bank", name="projr_ps")
                    for c in range(KD):
                        nc.tensor.matmul(
                            psr,
                            wst[c][:, ds(wroff, 128)],
                            xB[c][qb],
                            start=(c == 0),
                            stop=(c == KD - 1),
                        )
                    m1 = wp.tile([128, 512], BF16, tag="m1", bufs=3, name="m1")
                    nc.vector.tensor_mul(m1, ps, cosT[:, ds(512 * qb, 512)])
                    m2 = wp.tile([128, 512], BF16, tag="m2", bufs=3, name="m2")
                    nc.vector.tensor_mul(m2, psr, sinST[:, ds(512 * qb, 512)])
                    nc.vector.tensor_add(outT[:, ds(512 * qb, 512)], m1, m2)

            v_aug = [None] * NCH

            def emit_v(qb):
                # vT [vdim, pos] for this 512-block, then 4 transposes
                vt_ps = pp.tile([128, 512], F32, tag="bank", name="vt_ps")
                for c in range(KD):
                    nc.tensor.matmul(
                        vt_ps,
                        wst[c][:, ds(WV, 128)],
                        xB[c][qb],
                        start=(c == 0),
                        stop=(c == KD - 1),
                    )
                vt_sb = wp.tile([128, 512], BF16, tag="vt_sb", bufs=3,
                                name="vt_sb")
                nc.scalar.copy(out=vt_sb, in_=vt_ps)
                for sub in range(4):
                    ch = 4 * qb + sub
                    va = cp.tile([128, 130], BF16, tag=f"v_aug{ch}",
                                 name=f"v_aug{ch}")
                    vag = va.rearrange("p (g c) -> p g c", c=65)
                    nc.vector.memset(vag[:, :, 64], 1.0)
                    tp = pps.tile([128, 128], BF16, tag="small", name="v_tp")
                    nc.tensor.transpose(tp, vt_sb[:, ds(128 * sub, 128)],
                                        ident_bf)
                    if sub % 2 == 0:
                        nc.vector.tensor_copy(out=va[:, 0:64], in_=tp[:, 0:64])
                        nc.vector.tensor_copy(out=va[:, 65:129],
                                              in_=tp[:, 64:128])
                    else:
                        nc.scalar.copy(out=va[:, 0:64], in_=tp[:, 0:64])
                        nc.scalar.copy(out=va[:, 65:129], in_=tp[:, 64:128])
                    v_aug[ch] = va

            k_cT = cp.tile([128, LC], BF16, tag="k_cT")
            vc_aug = [None] * 4

            def emit_kcvc():
                kc_ps = pp.tile([128, LC], F32, tag="bank", name="kc_ps")
                for d in range(KD):
                    nc.tensor.matmul(
                        kc_ps,
                        wst[d][:, ds(WKC, 128)],
                        x_cT[d],
                        start=(d == 0),
                        stop=(d == KD - 1),
                    )
                nc.vector.tensor_copy(out=k_cT, in_=kc_ps)
                vc_ps = pp.tile([128, LC], F32, tag="bank", name="vc_ps")
                for d in range(KD):
                    nc.tensor.matmul(
                        vc_ps,
                        wst[d][:, ds(WVC, 128)],
                        x_cT[d],
                        start=(d == 0),
                        stop=(d == KD - 1),
                    )
                v_cT = wp.tile([128, LC], BF16, tag="v_cT", bufs=1,
                               name="v_cT")
                nc.vector.tensor_copy(out=v_cT, in_=vc_ps)
                for ch in range(4):
                    wlen = min(128, LC - 128 * ch)  # 128,128,128,127
                    va = cp.tile([128, 130], BF16, tag=f"vc_aug{ch}",
                                 name=f"vc_aug{ch}")
                    vag = va.rearrange("p (g c) -> p g c", c=65)
                    nc.vector.memset(vag[:, :, 64], 1.0)
                    tp = pps.tile([128, 128], BF16, tag="small", name="vc_tp")
                    nc.tensor.transpose(tp[0:wlen, :],
                                        v_cT[:, ds(128 * ch, wlen)], ident_bf)
                    nc.scalar.copy(out=va[0:wlen, 0:64], in_=tp[0:wlen, 0:64])
                    nc.scalar.copy(out=va[0:wlen, 65:129],
                                   in_=tp[0:wlen, 64:128])
                    vc_aug[ch] = va

            def emit_scores(qb):
                exc = [[None] * (qb + 1) for _ in range(2)]
                exw = [[None, None] for _ in range(2)]
                brs = [None, None]
                for h in range(2):
                    hs = 64 * h
                    qs = qT[ds(hs, 64), ds(512 * qb, 512)]
                    for wc in range(qb + 1):
                        wlen = min(128, LC - 128 * wc)
                        sc = pp.tile([128, 512], F32, tag="bank", name="sc_ps")
                        nc.tensor.matmul(
                            sc[0:wlen, :],
                            k_cT[ds(hs, 64), ds(128 * wc, wlen)],
                            qs,
                            start=True,
                            stop=True,
                        )
                        ex = wp.tile([128, 512], BF16, tag="exc", bufs=20,
                                     name="exc")
                        if wc >= qb - 1:
                            exr = wp.tile([128, 512], BF16, tag="exr", bufs=4,
                                          name="exr")
                            nc.scalar.activation(
                                out=exr[0:wlen, :], in_=sc[0:wlen, :],
                                func=AF.Exp, scale=0.125,
                            )
                            msk = md0 if wc == qb else md1
                            nc.vector.tensor_mul(
                                ex[0:wlen, :], exr[0:wlen, :], msk[0:wlen, :]
                            )
                        else:
                            nc.scalar.activation(
                                out=ex[0:wlen, :], in_=sc[0:wlen, :],
                                func=AF.Exp, scale=0.125,
                            )
                        exc[h][wc] = ex

                    for p in range(2):
                        kc0 = 4 * qb + 2 * p
                        ncols = 512 if kc0 < 14 else 384
                        wps = pp.tile([128, 512], F32, tag="bank", name="win_ps")
                        for j in range(2):
                            kc = kc0 + j
                            qcols = min(256, L - 128 * kc)
                            nc.tensor.matmul(
                                wps[:, ds(256 * j, qcols)],
                                kT[ds(hs, 64), ds(128 * kc, 128)],
                                qT[ds(hs, 64), ds(128 * kc, qcols)],
                                start=True,
                                stop=True,
                                skip_group_check=True,
                            )
                        ewr = wp.tile([128, 512], BF16, tag="ewr", bufs=4,
                                      name="ewr")
                        nc.scalar.activation(
                            out=ewr[:, 0:ncols], in_=wps[:, 0:ncols],
                            func=AF.Exp, scale=0.125,
                        )
                        ew = wp.tile([128, 512], BF16, tag="exw", bufs=16,
                                     name="exw")
                        nc.vector.tensor_mul(
                            ew[:, 0:ncols], ewr[:, 0:ncols], mwin[:, 0:ncols]
                        )
                        exw[h][p] = ew

                    if qb >= 1:
                        # bridge: keys chunk 4qb-1 vs q-chunk 4qb (replaces
                        # the cross-qb prev_exw handoff)
                        kc = 4 * qb - 1
                        bps = pps.tile([128, 128], F32, tag="small", name="bps")
                        nc.tensor.matmul(
                            bps,
                            kT[ds(hs, 64), ds(128 * kc, 128)],
                            qT[ds(hs, 64), ds(128 * (kc + 1), 128)],
                            start=True,
                            stop=True,
                        )
                        bre = wp.tile([128, 128], BF16, tag="bre", bufs=3,
                                      name="bre")
                        nc.scalar.activation(
                            out=bre, in_=bps, func=AF.Exp, scale=0.125,
                        )
                        br = wp.tile([128, 128], BF16, tag="br", bufs=6,
                                     name="br")
                        nc.vector.tensor_mul(br, bre, mwin[:, 128:256])
                        brs[h] = br
                return exc, exw, brs

            def emit_av_mms(qb, sub, exc, exw, brs):
                c = 4 * qb + sub
                av = pps.tile([128, 130], F32, tag="small", name="av")
                nc.tensor.matmul(
                    av, onesrow, sinkrow, start=True, stop=False,
                    skip_group_check=True,
                )
                for h in range(2):
                    last_h = (h == 1)
                    cols = ds(65 * h, 65)
                    pcur, jcur = sub // 2, sub % 2
                    nc.tensor.matmul(
                        av[:, cols],
                        exw[h][pcur][:, ds(256 * jcur, 128)],
                        v_aug[c][:, cols],
                        start=False, stop=False,
                        skip_group_check=True,
                    )
                    if c > 0:
                        if sub == 0:
                            epv = brs[h][:, 0:128]
                        else:
                            ew_, jpv = exw[h][(sub - 1) // 2], (sub - 1) % 2
                            epv = ew_[:, ds(256 * jpv + 128, 128)]
                        nc.tensor.matmul(
                            av[:, cols],
                            epv,
                            v_aug[c - 1][:, cols],
                            start=False, stop=False,
                            skip_group_check=True,
                        )
                    for wc in range(qb + 1):
                        wlen = min(128, LC - 128 * wc)
                        nc.tensor.matmul(
                            av[:, cols],
                            exc[h][wc][0:wlen, ds(128 * sub, 128)],
                            vc_aug[wc][0:wlen, cols],
                            start=False,
                            stop=(last_h and wc == qb),
                            skip_group_check=True,
                        )
                return av

            outsb = [None] * NB

            def emit_av_chain(av, c):
                avg = av.rearrange("p (g c) -> p g c", c=65)
                rec2 = wp.tile([128, 2], F32, tag="rec2", bufs=4, name="rec2")
                nc.vector.reciprocal(out=rec2, in_=avg[:, :, 64])
                avn = wp.tile([128, 128], BF16, tag="avn", bufs=4, name="avn")
                for h in range(2):
                    nc.vector.tensor_scalar(
                        out=avn[:, ds(64 * h, 64)],
                        in0=av[:, ds(65 * h, 64)],
                        scalar1=rec2[:, ds(h, 1)],
                        scalar2=None,
                        op0=ALU.mult,
                    )
                trp = pps.tile([128, 128], BF16, tag="small", name="trp")
                nc.tensor.transpose(trp, avn, ident_bf)
                avT = wp.tile([128, 128], BF16, tag="avT", bufs=4, name="avT")
                if c % 2 == 0:
                    nc.vector.tensor_copy(out=avT, in_=trp)
                else:
                    nc.scalar.copy(out=avT, in_=trp)
                po = pp.tile([128, 512], F32, tag="bank", name="po")
                nc.tensor.matmul(po, avT, woT, start=True, stop=True)
                qb, sub = c // 4, c % 4
                if sub == 0:
                    outsb[qb] = wp.tile([128, 2048], BF16, tag="osb", bufs=3,
                                        name="osb")
                dst = outsb[qb][:, ds(512 * sub, 512)]
                if sub % 2 == 0:
                    nc.scalar.copy(out=dst, in_=po)
                else:
                    nc.vector.tensor_copy(out=dst, in_=po)
                if sub == 3:
                    nc.sync.dma_start(
                        out=outp_d.rearrange("(b s p) f -> p b s f",
                                             b=NB, s=4)[:, qb],
                        in_=outsb[qb],
                    )

            def emit_av_block(qb, exc, exw, brs):
                avs = [None] * 4
                for sub in range(4):
                    avs[sub] = emit_av_mms(qb, sub, exc, exw, brs)
                    if sub >= 2:
                        emit_av_chain(avs[sub - 2], 4 * qb + sub - 2)
                emit_av_chain(avs[2], 4 * qb + 2)
                emit_av_chain(avs[3], 4 * qb + 3)

            # ---------------- main emission (qb-major) -------------------
            emit_kcvc()
            emit_proj_qk(0)
            emit_v(0)
            tiles = {}
            for qb in range(NB):
                tiles[qb] = emit_scores(qb)
                if qb >= 1:
                    exc, exw, brs = tiles.pop(qb - 1)
                    emit_av_block(qb - 1, exc, exw, brs)
                if qb + 1 < NB:
                    emit_proj_qk(qb + 1)
                    emit_v(qb + 1)
            exc, exw, brs = tiles.pop(NB - 1)
            emit_av_block(NB - 1, exc, exw, brs)

    nc.compile()
    return nc


def _host_prep(inputs):
    """Build the 8 per-core input maps from full inputs."""
    import ml_dtypes

    bf = ml_dtypes.bfloat16
    x = np.asarray(inputs["x"], dtype=np.float32)
    wq = np.asarray(inputs["wq"], dtype=np.float32)
    wk = np.asarray(inputs["wk"], dtype=np.float32)
    wv = np.asarray(inputs["wv"], dtype=np.float32)
    wo = np.asarray(inputs["wo"], dtype=np.float32)
    wk_c = np.asarray(inputs["wk_c"], dtype=np.float32)
    wv_c = np.asarray(inputs["wv_c"], dtype=np.float32)
    gate_logits = np.asarray(inputs["gate_logits"], dtype=np.float32)
    sink_logit = np.asarray(inputs["sink_logit"], dtype=np.float32)

    # rope tables
    half = HD // 2
    inv_freq = 1.0 / (THETA ** (np.arange(half, dtype=np.float32) / half))
    t = np.arange(L, dtype=np.float32)
    f = t[:, None] * inv_freq[None, :]  # [L, 32]
    cos32 = np.cos(f).T.astype(np.float32)  # [32, L]
    sin32 = np.sin(f).T.astype(np.float32)
    cosT = np.tile(cos32, (4, 1))  # rows: i%32
    sinST = np.concatenate([-sin32, sin32, -sin32, sin32], axis=0)

    # masks
    qr = np.arange(512)
    kr = np.arange(128)
    low = (qr[None, 0:128] >= kr[:, None]).astype(np.float32)   # cur chunk
    upp = (kr[:, None] > qr[None, 0:128]).astype(np.float32)    # prev chunk
    mwin = np.concatenate([low, upp, low, upp], axis=1)  # [128, 512]
    md0 = (qr[None, :] >= 4 * kr[:, None] + 7).astype(np.float32)
    md1 = (qr[None, :] >= 4 * kr[:, None] - 505).astype(np.float32)

    mega = np.concatenate(
        [cosT, sinST, mwin, md0, md1, np.zeros((128, 512), np.float32)],
        axis=1,
    )  # [128, 6144]; wo slice filled per core

    # compressed pooling on host: x_c[b, w] = sum_r g_r x[b, 4w+r]
    g = np.exp(gate_logits - gate_logits.max())
    g = (g / g.sum()).astype(np.float32)
    idx = np.arange(RATIO)[None, :] + np.arange(LC)[:, None] * STRIDE  # [LC, 8]
    x_c = np.einsum("bwrd,r->bwd", x[:, idx, :], g)  # [B, LC, D]

    # rope permutation of projection output dims (within the 128-dim slice):
    # row j of the rotated weights = row swap(j): 0..31 <-> 32..63 per head
    perm = np.arange(128).reshape(2, 2, 32)[:, ::-1, :].reshape(128)

    in_maps = []
    for core in range(NCORES):
        b, grp = divmod(core, 4)
        sl = slice(128 * grp, 128 * (grp + 1))
        m = mega.copy()
        m[:, WOT:WOT + 512] = wo[:, sl].T
        wq_s = wq[sl]
        wk_s = wk[sl]
        wst = np.concatenate(
            [wq_s.T, wq_s[perm].T, wk_s.T, wk_s[perm].T,
             wv[sl].T, wk_c[sl].T, wv_c[sl].T], axis=1
        )  # [512, 896]
        sinkrow = np.zeros((1, 130), np.float32)
        sinkrow[0, 64] = np.exp(sink_logit[2 * grp, 0])
        sinkrow[0, 129] = np.exp(sink_logit[2 * grp + 1, 0])
        in_maps.append(
            {
                "xT": np.ascontiguousarray(x[b].T).astype(bf),
                "xcT": np.ascontiguousarray(x_c[b].T).astype(bf),
                "wst": np.ascontiguousarray(wst).astype(bf),
                "mega": m.astype(bf),
                "sinkrow": sinkrow.astype(bf),
            }
        )
    return in_maps


def kernel(**inputs) -> np.ndarray:
    from concourse.bass_utils import run_bass_kernel_spmd

    if "nc" not in _CACHE:
        _CACHE["nc"] = _build_nc()
    nc = _CACHE["nc"]

    in_maps = _host_prep(inputs)
    res = run_bass_kernel_spmd(nc, in_maps, core_ids=list(range(NCORES)))
    out = np.zeros((B, L, D), dtype=np.float32)
    for core in range(NCORES):
        b = core // 4
        out[b] += np.asarray(res.results[core]["outp"], dtype=np.float32)
    return out
